# revision 44
# baseline (speedup 1.0000x reference)
# Trainium2 Bass kernel for KNN-style sparse cross-attention (v6).
#
# reference semantics:
#   q  = src @ w_src.T                          [B,S,D]
#   kv = tgt @ w_tgt.T                          [B,S,T,2D]
#   attn[b,h,s,t] = <q[b,s,h], k[b,s,t,h]> / sqrt(DH)
#   softmax over t (with padding mask; fully-masked queries output 0)
#   out = (attn @ v) @ out_proj.T
#
# Structure: contract over t BEFORE projecting with Wv (kills the big
# v = tgt @ Wv^T projection over all 65536 kv rows):
#   q_eff = src Wq^T Wk / sqrt(DH)   (K folded away, rank-64 two-stage)
#   scores TRANSPOSED [st, (g,h,q')], st = kv-row on partitions,
#     4 queries x 32 t per 128-row group, 32-col matmuls.
#   softmax: exp on scalar; masked column sums via one-hot stationaries
#     (+BIG accumulated into wrong (row-group, query) slots so 1/sum ~ 0
#     kills cross-query garbage; +eps on the diagonal handles
#     fully-masked queries); reciprocal batched over block PAIRS on
#     vector; broadcast back over partitions by a 4-row matmul.
#     Invalid kv rows are ZEROED in tgn on the host.
#   ctx[d, (g,h,q')] = A^T tgt  (tgt natural-layout stationary)
#   out_av = ctx @ Wv_h^T per head; out = out_av @ Wo^T (flipped).
#
# All DRAM operands are HOST-PRE-SWIZZLED so each SBUF tile loads as one
# contiguous multi-KB run per partition (few DMA descriptor rows).
# Emission is software-pipelined: QK(k) | sums(k-1) | recip | A/ctx(k-3).
import os
from contextlib import ExitStack

import numpy as np

import concourse.bacc as bacc
import concourse.mybir as mybir
import concourse.tile as tile
from concourse import bass_utils

N_CORES = 8
D = 512          # d_model
H = 8            # heads
DH = 64          # head dim
T = 32           # KNN set size per query
BS = 2048        # B*S total queries
R = BS // N_CORES     # queries per core (256)
RT = R * T            # kv rows per core (8192)
PT = 128              # partition tile
KD = D // PT          # 4 contraction tiles over d_model
QB = 16               # queries per block
NB = R // QB          # 16 blocks per core
G = 4                 # query groups per block (4 queries x 32 t = 128 st)
W = 1024              # kv rows per superchunk (2 blocks)
NSUP = RT // W        # 8 superchunks
BIG = 60000.0         # garbage-sum offset (f16-representable)

F32 = mybir.dt.float32
F16 = mybir.dt.float16
ACTF = mybir.ActivationFunctionType


def build_program(n_cores=N_CORES):
    mdt = F16

    nc = bacc.Bacc(
        "TRN2",
        target_bir_lowering=False,
        debug=False,
        enable_asserts=False,
        num_devices=n_cores,
    )

    # ph1 = src(1024) | ws(2048); ph2 = wk(2048) | dsb(256) | auxrows(384);
    # ph3 = wv(2048) | wo(2048).  aux rows 0:4 = b4f[128] | oh4[128] | anti4[4].
    ph1 = nc.dram_tensor("ph1", [PT, 3072], mdt, kind="ExternalInput").ap()
    ph2 = nc.dram_tensor("ph2", [PT, 2952], mdt, kind="ExternalInput").ap()
    ph3 = nc.dram_tensor("ph3", [PT, 4096], mdt, kind="ExternalInput").ap()
    tgTw = nc.dram_tensor("tgTw", [PT, NSUP * KD * W], mdt, kind="ExternalInput").ap()
    tgNw = nc.dram_tensor("tgNw", [PT, NSUP * KD * W], mdt, kind="ExternalInput").ap()
    outn = nc.dram_tensor("outn", [R, D], F16, kind="ExternalOutput").ap()

    lp = nc.allow_low_precision("fp32 PSUM accumulation, 16-bit stores")
    lp.__enter__()
    with tile.TileContext(nc) as tc, ExitStack() as ctx:
        consts = ctx.enter_context(tc.tile_pool(name="consts", bufs=1))
        io_t = ctx.enter_context(tc.tile_pool(name="io_t", bufs=8))
        io_n = ctx.enter_context(tc.tile_pool(name="io_n", bufs=8))
        one = ctx.enter_context(tc.tile_pool(name="one", bufs=1))
        blkp = ctx.enter_context(tc.tile_pool(name="blkp", bufs=7))
        work = ctx.enter_context(tc.tile_pool(name="work", bufs=2))
        ps_scr = ctx.enter_context(tc.tile_pool(name="ps_scr", bufs=2, space="PSUM"))
        ps_sq = ctx.enter_context(tc.tile_pool(name="ps_sq", bufs=3, space="PSUM"))

        # ---- phase 1 DMAs: what qeff needs ----
        p1_sb = consts.tile([PT, 3072], mdt, name="p1_sb")
        nc.sync.dma_start(p1_sb, ph1)
        src_sb = p1_sb[:, 0:KD * R]
        ws_sb = p1_sb[:, KD * R : KD * R + KD * D]
        p2_sb = consts.tile([PT, 2952], mdt, name="p2_sb")
        nc.sync.dma_start(p2_sb, ph2)
        wk_sb = p2_sb[:, 0:2048]
        ds_sb = p2_sb[:, 2048:2304]
        b4f512 = p2_sb[0:4, 2304:2816]
        oh4 = p2_sb[0:4, 2816:2944]
        anti4 = p2_sb[0:4, 2944:2948]

        # ---- phase 2: tgt superchunk prefetch ----
        tgTs = {}
        tgNs = {}

        def fetch_sup(sc):
            tgT = io_t.tile([PT, KD * W], mdt, name="tgT")
            nc.sync.dma_start(tgT, tgTw[:, sc * KD * W : (sc + 1) * KD * W])
            tgTs[sc] = tgT
            tgN = io_n.tile([PT, KD * W], mdt, name="tgN")
            nc.sync.dma_start(tgN, tgNw[:, sc * KD * W : (sc + 1) * KD * W])
            tgNs[sc] = tgN

        for sc in range(min(4, NSUP)):
            fetch_sup(sc)

        # ---- phase 3: tail weights, then the rest of tgt ----
        p3_sb = consts.tile([PT, 4096], mdt, name="p3_sb")
        nc.sync.dma_start(p3_sb, ph3)
        wv_sb = p3_sb[:, 0:KD * D]
        wo_sb = p3_sb[:, KD * D : 2 * KD * D]
        for sc in range(4, NSUP):
            fetch_sup(sc)

        # ---- q_eff: [128, j(4) x h(8) x s(256)]; the (h,q') gather for
        # QK happens in the matmul moving AP, copies stay contiguous.
        qeff = one.tile([PT, KD * H * R], mdt, name="qeff")
        qeff4 = qeff.rearrange("p (j h s) -> p j h s", j=KD, h=H)
        q_sb = one.tile([PT, KD * R], mdt, name="q_sb")

        def qeff_stage():
            with tc.tile_pool(name="ps_q", bufs=3, space="PSUM") as ps_q:
                qslots = [ps_q.tile([PT, 2 * R], F32, name="qs") for _ in range(3)]

                def qslot(i):
                    return qslots[(i // 2) % 3][:, (i % 2) * R : (i % 2 + 1) * R]

                for m in range(KD):
                    qpp = qslot(m)
                    for j in range(KD):
                        nc.tensor.matmul(
                            qpp,
                            ws_sb[:, j * D + m * PT : j * D + (m + 1) * PT],
                            src_sb[:, j * R : (j + 1) * R],
                            start=(j == 0),
                            stop=(j == KD - 1),
                        )
                    if m % 2 == 1:
                        (nc.scalar.copy if m == 1 else nc.vector.tensor_copy)(
                            q_sb[:, (m - 1) * R : (m + 1) * R],
                            qslots[(m // 2) % 3],
                        )
                for h in range(H):
                    p0 = (h % 2) * 64
                    for c in range(KD):
                        i = h * KD + c
                        qp = qslot(i)
                        nc.tensor.matmul(
                            qp,
                            wk_sb[p0 : p0 + 64,
                                  (h // 2) * D + c * PT : (h // 2) * D + (c + 1) * PT],
                            q_sb[p0 : p0 + 64, (h // 2) * R : (h // 2 + 1) * R],
                            start=True,
                            stop=True,
                        )
                        if c % 2 == 1:
                            (nc.scalar.copy if (i // 2) % 2 == 0
                             else nc.vector.tensor_copy)(
                                qeff4[:, c - 1 : c + 1, h, :],
                                qslots[(i // 2) % 3],
                            )

        qeff_stage()
        ps_rb = ctx.enter_context(tc.tile_pool(name="ps_rb", bufs=1, space="PSUM"))
        ps_ctx = ctx.enter_context(tc.tile_pool(name="ps_ctx", bufs=2, space="PSUM"))

        # ctx layout: [p, blk(16) x dc(4) x (g,h,q')(128)] -> contiguous
        # per-block copies; the tail matmul gathers (dc, h) slices via AP.
        ctx_sb = one.tile([PT, NB * KD * PT], mdt, name="ctx_sb")
        ctx6 = ctx_sb.rearrange(
            "p (b j g h q) -> p b j g h q", b=NB, j=KD, g=G, h=H
        )
        oav_sb = one.tile([PT, KD * R], mdt, name="oav_sb")

        # group (4 blocks = 1 quad) pipeline state
        scrs = {}
        ems = {}
        sqs = {}
        rcs = {}
        rbs = {}
        abs_ = {}

        def stage1(gp):
            """QK matmuls for 4 blocks + one exp."""
            scr = ps_scr.tile([PT, 512], F32, name="scr")
            scrs[gp] = scr
            for bi in range(4):
                blk = gp * 4 + bi
                sc = blk // 2
                bl = blk % 2
                tgT = tgTs[sc].rearrange("p (j m) -> p j m", j=KD)
                for g in range(G):
                    for j in range(KD):
                        nc.tensor.matmul(
                            scr[:, bi * 128 + g * 32 : bi * 128 + (g + 1) * 32],
                            tgT[:, j, bl * 512 + g * PT : bl * 512 + (g + 1) * PT],
                            qeff4[:, j, :, blk * QB + g * 4 : blk * QB + g * 4 + 4],
                            start=(j == 0),
                            stop=(j == KD - 1),
                        )
            em = blkp.tile([PT, 512], mdt, name="em")
            nc.scalar.activation(em, scr, ACTF.Exp)
            ems[gp] = em

        def stage2a(gp):
            """one anti matmul + 16 masked column sums."""
            em = ems[gp]
            sq = ps_sq.tile([4, 512], F32, name="sq")
            sqs[gp] = sq
            nc.tensor.matmul(sq, anti4, b4f512, start=True, stop=False,
                             skip_group_check=True)
            for bi in range(4):
                blk = gp * 4 + bi
                for g in range(G):
                    nc.tensor.matmul(
                        sq[:, bi * 128 + g * 32 : bi * 128 + (g + 1) * 32],
                        ds_sb[:, (blk * G + g) * 4 : (blk * G + g) * 4 + 4],
                        em[:, bi * 128 + g * 32 : bi * 128 + (g + 1) * 32],
                        start=False,
                        stop=True,
                        skip_group_check=True,
                    )

        def stage2b(gp):
            """one reciprocal per group."""
            rc = blkp.tile([4, 512], mdt, name="rc")
            nc.vector.reciprocal(rc, sqs[gp])
            rcs[gp] = rc

        def stage2c(gp):
            """one broadcast matmul + one A-mult per group."""
            rb = ps_rb.tile([PT, 512], F32, name="rb")
            rbs[gp] = rb
            nc.tensor.matmul(rb, oh4, rcs[gp], start=True, stop=True)
            ab = blkp.tile([PT, 512], mdt, name="ab")
            nc.vector.tensor_mul(ab, ems[gp], rb)
            abs_[gp] = ab

        def stage3(gp):
            """ctx matmuls + PSUM->SBUF copies, per block."""
            ab = abs_[gp]
            for bi in range(4):
                blk = gp * 4 + bi
                sc = blk // 2
                bl = blk % 2
                tgN = tgNs[sc].rearrange("p (c d) -> p c d", c=W // PT)
                cp = ps_ctx.tile([PT, 512], F32, name="cp", tag="cp")
                for dc in range(KD):
                    for g in range(G):
                        nc.tensor.matmul(
                            cp[:, dc * PT + g * 32 : dc * PT + (g + 1) * 32],
                            tgN[:, bl * G + g, dc * PT : (dc + 1) * PT],
                            ab[:, bi * 128 + g * 32 : bi * 128 + (g + 1) * 32],
                            start=True,
                            stop=True,
                        )
                nc.scalar.copy(ctx_sb[:, blk * 512 : (blk + 1) * 512], cp)

        def do_tail(half):
            # project ctx -> out_av for 128 queries (8 blocks), then out.
            for h in range(H):
                ovp = ps_ctx.tile([64, PT], F32, name="ovp", tag="cp")
                for dc in range(KD):
                    mov = ctx6[:, half * 8 : (half + 1) * 8, dc, :, h, :]
                    nc.tensor.matmul(
                        ovp,
                        wv_sb[:, dc * D + h * DH : dc * D + (h + 1) * DH],
                        mov,
                        start=(dc == 0),
                        stop=(dc == KD - 1),
                    )
                p0 = (h % 2) * 64
                (nc.scalar.copy if h % 2 == 0 else nc.vector.tensor_copy)(
                    oav_sb[p0 : p0 + 64,
                           (h // 2) * R + half * PT : (h // 2) * R + (half + 1) * PT],
                    ovp,
                )
            op = ps_ctx.tile([PT, D], F32, name="op", tag="cp")
            for hh in range(KD):
                nc.tensor.matmul(
                    op,
                    oav_sb[:, hh * R + half * PT : hh * R + (half + 1) * PT],
                    wo_sb[:, hh * D : (hh + 1) * D],
                    start=(hh == 0),
                    stop=(hh == KD - 1),
                )
            res = work.tile([PT, D], mdt, name="res")
            nc.scalar.copy(res, op)
            nc.sync.dma_start(outn[half * PT : (half + 1) * PT, :], res)

        # software-pipelined group emission: A-mults emitted before the
        # reciprocal each step so the rb-bank recycle never sits behind a
        # multi-us reciprocal in the vector queue.
        NG = NB // 4
        for s in range(NG + 4):
            if s < NG:
                stage1(s)
            if 1 <= s <= NG:
                stage2a(s - 1)
                stage2b(s - 1)
            if 4 <= s:
                g3 = s - 4
                stage3(g3)
                if g3 == 1:
                    do_tail(0)
            if 3 <= s and s - 3 < NG:
                stage2c(s - 3)
        do_tail(1)

    lp.__exit__(None, None, None)
    nc.compile()
    return nc


_PROGRAM = None


def _get_program():
    global _PROGRAM
    if _PROGRAM is None:
        _PROGRAM = build_program()
    return _PROGRAM


def prep_inputs(src, tgt, tgt_padding_mask, in_proj_weight, in_proj_bias,
                out_proj_weight, out_proj_bias):
    """Host-side shard + swizzled layout prep. Returns per-core in_maps."""
    mnp = np.float16
    f32 = np.float32
    src2 = np.asarray(src, dtype=f32).reshape(BS, D)
    tgt2 = np.asarray(tgt, dtype=f32).reshape(BS * T, D)
    mask2 = np.asarray(tgt_padding_mask).astype(bool).reshape(BS, T)
    wm = np.asarray(in_proj_weight, dtype=f32)
    wo = np.asarray(out_proj_weight, dtype=f32)
    Wq, Wk, Wv = wm[:D], wm[D : 2 * D], wm[2 * D :]

    def sw(mat):  # [512, M] row-chunked -> [128, KD*M] per-partition runs
        M = mat.shape[1]
        return np.ascontiguousarray(
            mat.reshape(KD, PT, M).transpose(1, 0, 2).reshape(PT, KD * M)
        ).astype(mnp)

    scl = f32(1.0 / np.sqrt(DH))
    wsw = sw((Wq * scl).T)
    wk2 = np.ascontiguousarray(
        Wk.reshape(H // 2, 2, DH, D).transpose(1, 2, 0, 3).reshape(PT, (H // 2) * D)
    ).astype(mnp)
    wvw = sw(Wv.T)
    wow = sw(wo.T)

    # aux consts (packed into ph2 rows 0:4)
    auxr = np.zeros((PT, 648), dtype=mnp)
    cc5 = np.arange(512)
    cc = np.arange(PT)
    auxr[0:4, 0:512] = (cc5[None, :] % 4 == np.arange(4)[:, None])     # b4f512
    auxr[0:4, 512:640] = (cc[None, :] // 32 == np.arange(4)[:, None])  # oh4
    auxr[0:4, 640:644] = BIG * (1.0 - np.eye(4, dtype=f32)) + 1e-4 * np.eye(4, dtype=f32)
    ph3 = np.ascontiguousarray(np.concatenate([wvw, wow], axis=1))

    valid_all = ~mask2                                              # [BS, T]
    tgt16 = tgt2.astype(mnp)
    tgt16[~valid_all.reshape(-1)] = 0                               # zero invalid kv rows

    pp = np.arange(PT)
    in_maps = []
    for c in range(N_CORES):
        rows = slice(c * R, (c + 1) * R)
        kvrows = slice(c * RT, (c + 1) * RT)
        tkv = tgt2[kvrows].astype(mnp)                              # [RT, 512]
        tkz = tgt16[kvrows]                                         # zeroed
        # tgTw[p, s, j, m] = tkv[s*W + m, j*128 + p]
        tgTw = np.ascontiguousarray(
            tkv.reshape(NSUP, W, KD, PT).transpose(3, 0, 2, 1).reshape(PT, -1))
        # tgNw[p, s, cch, d] = tkz[s*W + cch*128 + p, d]
        tgNw = np.ascontiguousarray(
            tkz.reshape(NSUP, W // PT, PT, D).transpose(2, 0, 1, 3).reshape(PT, -1))
        srcw = sw(src2[rows].T)
        valid = valid_all[rows]                                     # [R, T]
        # dsb[p, blk, g, k] = [p//32==k] * valid[blk*16+g*4+k, p%32]
        vg = valid.reshape(NB, G, 4, T)                             # [blk,g,q'',t]
        dsbm = np.zeros((PT, NB, G, 4), dtype=mnp)
        for k in range(4):
            sel = pp // 32 == k
            dsbm[sel, :, :, k] = vg[:, :, k, :].transpose(2, 0, 1)[pp[sel] % 32]
        ph1 = np.ascontiguousarray(np.concatenate([srcw, wsw], axis=1))
        ph2 = np.ascontiguousarray(np.concatenate(
            [wk2, dsbm.reshape(PT, NB * G * 4), auxr], axis=1))
        in_maps.append({
            "ph1": ph1, "ph2": ph2, "ph3": ph3,
            "tgTw": tgTw,
            "tgNw": tgNw,
        })
    return in_maps


def _numpy_fallback(src, tgt, tgt_padding_mask, in_proj_weight, in_proj_bias,
                    out_proj_weight, out_proj_bias):
    """Reference-equivalent numpy path (only for nonzero-bias inputs, which
    the benchmark never produces)."""
    B, S, _ = src.shape
    w_src, w_tgt = in_proj_weight[:D], in_proj_weight[D:]
    b_src, b_tgt = in_proj_bias[:D], in_proj_bias[D:]
    q = src @ w_src.T + b_src
    kv = tgt @ w_tgt.T + b_tgt
    k, v = kv[..., :D], kv[..., D:]
    inv = tgt_padding_mask.astype(bool)
    noval = inv.all(-1)
    inv = inv & ~noval[..., None]
    q = q.reshape(B, S, H, DH)
    k = k.reshape(B, S, T, H, DH)
    v = v.reshape(B, S, T, H, DH)
    att = np.einsum("bshd,bsthd->bhst", q, k)
    att = np.where(inv[:, None], -np.inf, att) / np.sqrt(DH)
    att = att - att.max(-1, keepdims=True)
    att = np.exp(att)
    att = att / att.sum(-1, keepdims=True)
    out = np.einsum("bhst,bsthd->bshd", att, v).reshape(B, S, D)
    out = out @ out_proj_weight.T + out_proj_bias
    return np.where(noval[..., None], 0.0, out).astype(np.float32)


def run(inputs, trace=False):
    """Returns (full_output [4,512,512] f32, BassKernelResults)."""
    in_maps = prep_inputs(**inputs)
    nc = _get_program()
    res = bass_utils.run_bass_kernel_spmd(
        nc, in_maps, core_ids=list(range(N_CORES)), trace=trace
    )
    out = np.empty((BS, D), dtype=np.float32)
    for c in range(N_CORES):
        out[c * R : (c + 1) * R] = res.results[c]["outn"].astype(np.float32)
    return out.reshape(4, 512, D), res


def kernel(**inputs):
    inputs = {k: np.asarray(v) for k, v in inputs.items()}
    if (np.any(inputs["in_proj_bias"]) or np.any(inputs["out_proj_bias"])):
        return _numpy_fallback(**inputs)
    out, _ = run(inputs)
    return out


# revision 45
# speedup vs baseline: 1.0281x; 1.0281x over previous
# Trainium2 Bass kernel for KNN-style sparse cross-attention (v6).
#
# reference semantics:
#   q  = src @ w_src.T                          [B,S,D]
#   kv = tgt @ w_tgt.T                          [B,S,T,2D]
#   attn[b,h,s,t] = <q[b,s,h], k[b,s,t,h]> / sqrt(DH)
#   softmax over t (with padding mask; fully-masked queries output 0)
#   out = (attn @ v) @ out_proj.T
#
# Structure: contract over t BEFORE projecting with Wv (kills the big
# v = tgt @ Wv^T projection over all 65536 kv rows):
#   q_eff = src Wq^T Wk / sqrt(DH)   (K folded away, rank-64 two-stage)
#   scores TRANSPOSED [st, (g,h,q')], st = kv-row on partitions,
#     4 queries x 32 t per 128-row group, 32-col matmuls.
#   softmax: exp on scalar; masked column sums via one-hot stationaries
#     (+BIG accumulated into wrong (row-group, query) slots so 1/sum ~ 0
#     kills cross-query garbage; +eps on the diagonal handles
#     fully-masked queries); reciprocal batched over block PAIRS on
#     vector; broadcast back over partitions by a 4-row matmul.
#     Invalid kv rows are ZEROED in tgn on the host.
#   ctx[d, (g,h,q')] = A^T tgt  (tgt natural-layout stationary)
#   out_av = ctx @ Wv_h^T per head; out = out_av @ Wo^T (flipped).
#
# All DRAM operands are HOST-PRE-SWIZZLED so each SBUF tile loads as one
# contiguous multi-KB run per partition (few DMA descriptor rows).
# Emission is software-pipelined: QK(k) | sums(k-1) | recip | A/ctx(k-3).
import os
from contextlib import ExitStack

import numpy as np

import concourse.bacc as bacc
import concourse.mybir as mybir
import concourse.tile as tile
from concourse import bass_utils

N_CORES = 8
D = 512          # d_model
H = 8            # heads
DH = 64          # head dim
T = 32           # KNN set size per query
BS = 2048        # B*S total queries
R = BS // N_CORES     # queries per core (256)
RT = R * T            # kv rows per core (8192)
PT = 128              # partition tile
KD = D // PT          # 4 contraction tiles over d_model
QB = 16               # queries per block
NB = R // QB          # 16 blocks per core
G = 4                 # query groups per block (4 queries x 32 t = 128 st)
W = 1024              # kv rows per superchunk (2 blocks)
NSUP = RT // W        # 8 superchunks
BIG = 60000.0         # garbage-sum offset (f16-representable)

F32 = mybir.dt.float32
F16 = mybir.dt.float16
ACTF = mybir.ActivationFunctionType


def build_program(n_cores=N_CORES):
    mdt = F16

    nc = bacc.Bacc(
        "TRN2",
        target_bir_lowering=False,
        debug=False,
        enable_asserts=False,
        num_devices=n_cores,
    )

    # ph1 = src(1024) | ws(2048); ph2 = wk(2048) | dsb(256) | auxrows(384);
    # ph3 = wv(2048) | wo(2048).  aux rows 0:4 = b4f[128] | oh4[128] | anti4[4].
    ph1 = nc.dram_tensor("ph1", [PT, 3072], mdt, kind="ExternalInput").ap()
    ph2 = nc.dram_tensor("ph2", [PT, 2952], mdt, kind="ExternalInput").ap()
    ph3 = nc.dram_tensor("ph3", [PT, 4096], mdt, kind="ExternalInput").ap()
    tgTw = nc.dram_tensor("tgTw", [PT, NSUP * KD * W], mdt, kind="ExternalInput").ap()
    tgNw = nc.dram_tensor("tgNw", [PT, NSUP * KD * W], mdt, kind="ExternalInput").ap()
    outn = nc.dram_tensor("outn", [R, D], F16, kind="ExternalOutput").ap()

    lp = nc.allow_low_precision("fp32 PSUM accumulation, 16-bit stores")
    lp.__enter__()
    with tile.TileContext(nc) as tc, ExitStack() as ctx:
        consts = ctx.enter_context(tc.tile_pool(name="consts", bufs=1))
        io_t = ctx.enter_context(tc.tile_pool(name="io_t", bufs=8))
        io_n = ctx.enter_context(tc.tile_pool(name="io_n", bufs=8))
        one = ctx.enter_context(tc.tile_pool(name="one", bufs=1))
        blkp = ctx.enter_context(tc.tile_pool(name="blkp", bufs=6))
        work = ctx.enter_context(tc.tile_pool(name="work", bufs=2))
        ps_scr = ctx.enter_context(tc.tile_pool(name="ps_scr", bufs=2, space="PSUM"))
        ps_sq = ctx.enter_context(tc.tile_pool(name="ps_sq", bufs=2, space="PSUM"))

        # ---- phase 1 DMAs: what qeff needs ----
        p1_sb = consts.tile([PT, 3072], mdt, name="p1_sb")
        nc.sync.dma_start(p1_sb, ph1)
        src_sb = p1_sb[:, 0:KD * R]
        ws_sb = p1_sb[:, KD * R : KD * R + KD * D]
        p2_sb = consts.tile([PT, 2952], mdt, name="p2_sb")
        nc.sync.dma_start(p2_sb, ph2)
        wk_sb = p2_sb[:, 0:2048]
        ds_sb = p2_sb[:, 2048:2304]
        b4f512 = p2_sb[0:4, 2304:2816]
        oh4 = p2_sb[0:4, 2816:2944]
        anti4 = p2_sb[0:4, 2944:2948]

        # ---- phase 2: tgt superchunk prefetch ----
        tgTs = {}
        tgNs = {}

        def fetch_sup(sc):
            tgT = io_t.tile([PT, KD * W], mdt, name="tgT")
            nc.sync.dma_start(tgT, tgTw[:, sc * KD * W : (sc + 1) * KD * W])
            tgTs[sc] = tgT
            tgN = io_n.tile([PT, KD * W], mdt, name="tgN")
            nc.sync.dma_start(tgN, tgNw[:, sc * KD * W : (sc + 1) * KD * W])
            tgNs[sc] = tgN

        for sc in range(min(4, NSUP)):
            fetch_sup(sc)

        # ---- phase 3: tail weights, then the rest of tgt ----
        p3_sb = consts.tile([PT, 4096], mdt, name="p3_sb")
        nc.sync.dma_start(p3_sb, ph3)
        wv_sb = p3_sb[:, 0:KD * D]
        wo_sb = p3_sb[:, KD * D : 2 * KD * D]
        for sc in range(4, NSUP):
            fetch_sup(sc)

        # ---- q_eff: [128, j(4) x h(8) x s(256)]; the (h,q') gather for
        # QK happens in the matmul moving AP, copies stay contiguous.
        qeff = one.tile([PT, KD * H * R], mdt, name="qeff")
        qeff4 = qeff.rearrange("p (j h s) -> p j h s", j=KD, h=H)
        q_sb = one.tile([PT, KD * R], mdt, name="q_sb")

        def qeff_stage():
            with tc.tile_pool(name="ps_q", bufs=3, space="PSUM") as ps_q:
                qslots = [ps_q.tile([PT, 2 * R], F32, name="qs") for _ in range(3)]

                def qslot(i):
                    return qslots[(i // 2) % 3][:, (i % 2) * R : (i % 2 + 1) * R]

                for m in range(KD):
                    qpp = qslot(m)
                    for j in range(KD):
                        nc.tensor.matmul(
                            qpp,
                            ws_sb[:, j * D + m * PT : j * D + (m + 1) * PT],
                            src_sb[:, j * R : (j + 1) * R],
                            start=(j == 0),
                            stop=(j == KD - 1),
                        )
                    if m % 2 == 1:
                        (nc.scalar.copy if m == 1 else nc.vector.tensor_copy)(
                            q_sb[:, (m - 1) * R : (m + 1) * R],
                            qslots[(m // 2) % 3],
                        )
                for h in range(H):
                    p0 = (h % 2) * 64
                    for c in range(KD):
                        i = h * KD + c
                        qp = qslot(i)
                        nc.tensor.matmul(
                            qp,
                            wk_sb[p0 : p0 + 64,
                                  (h // 2) * D + c * PT : (h // 2) * D + (c + 1) * PT],
                            q_sb[p0 : p0 + 64, (h // 2) * R : (h // 2 + 1) * R],
                            start=True,
                            stop=True,
                        )
                        if c % 2 == 1:
                            (nc.scalar.copy if (i // 2) % 2 == 0
                             else nc.vector.tensor_copy)(
                                qeff4[:, c - 1 : c + 1, h, :],
                                qslots[(i // 2) % 3],
                            )

        qeff_stage()
        ps_rb = ctx.enter_context(tc.tile_pool(name="ps_rb", bufs=1, space="PSUM"))
        ps_ctx = ctx.enter_context(tc.tile_pool(name="ps_ctx", bufs=3, space="PSUM"))

        # ctx layout: [p, blk(16) x dc(4) x (g,h,q')(128)] -> contiguous
        # per-block copies; the tail matmul gathers (dc, h) slices via AP.
        ctx_sb = one.tile([PT, NB * KD * PT], mdt, name="ctx_sb")
        ctx6 = ctx_sb.rearrange(
            "p (b j g h q) -> p b j g h q", b=NB, j=KD, g=G, h=H
        )
        oav_sb = one.tile([PT, KD * R], mdt, name="oav_sb")

        # group (4 blocks = 1 quad) pipeline state
        scrs = {}
        ems = {}
        sqs = {}
        rcs = {}
        rbs = {}
        abs_ = {}

        def stage1(gp):
            """QK matmuls for 4 blocks + one exp."""
            scr = ps_scr.tile([PT, 512], F32, name="scr")
            scrs[gp] = scr
            for bi in range(4):
                blk = gp * 4 + bi
                sc = blk // 2
                bl = blk % 2
                tgT = tgTs[sc].rearrange("p (j m) -> p j m", j=KD)
                for g in range(G):
                    for j in range(KD):
                        nc.tensor.matmul(
                            scr[:, bi * 128 + g * 32 : bi * 128 + (g + 1) * 32],
                            tgT[:, j, bl * 512 + g * PT : bl * 512 + (g + 1) * PT],
                            qeff4[:, j, :, blk * QB + g * 4 : blk * QB + g * 4 + 4],
                            start=(j == 0),
                            stop=(j == KD - 1),
                        )
            em = blkp.tile([PT, 512], mdt, name="em")
            nc.scalar.activation(em, scr, ACTF.Exp)
            ems[gp] = em

        def stage2a(gp, hf):
            """anti + masked column sums + reciprocal for one half-group."""
            em = ems[gp]
            if hf == 0:
                sqs[gp] = ps_sq.tile([4, 512], F32, name="sq")
                rcs[gp] = blkp.tile([4, 512], mdt, name="rc")
            sq = sqs[gp]
            nc.tensor.matmul(sq[:, hf * 256 : (hf + 1) * 256], anti4,
                             b4f512[:, hf * 256 : (hf + 1) * 256],
                             start=True, stop=False, skip_group_check=True)
            for bi in (2 * hf, 2 * hf + 1):
                blk = gp * 4 + bi
                for g in range(G):
                    nc.tensor.matmul(
                        sq[:, bi * 128 + g * 32 : bi * 128 + (g + 1) * 32],
                        ds_sb[:, (blk * G + g) * 4 : (blk * G + g) * 4 + 4],
                        em[:, bi * 128 + g * 32 : bi * 128 + (g + 1) * 32],
                        start=False,
                        stop=True,
                        skip_group_check=True,
                    )
            nc.vector.reciprocal(
                rcs[gp][:, hf * 256 : (hf + 1) * 256],
                sq[:, hf * 256 : (hf + 1) * 256])

        def stage2c(gp, hf):
            """broadcast matmul + A-mult for one half-group."""
            if hf == 0:
                rbs[gp] = ps_rb.tile([PT, 512], F32, name="rb")
                abs_[gp] = blkp.tile([PT, 512], mdt, name="ab")
            rb = rbs[gp]
            nc.tensor.matmul(rb[:, hf * 256 : (hf + 1) * 256], oh4,
                             rcs[gp][:, hf * 256 : (hf + 1) * 256],
                             start=True, stop=True)
            nc.vector.tensor_mul(
                abs_[gp][:, hf * 256 : (hf + 1) * 256],
                ems[gp][:, hf * 256 : (hf + 1) * 256],
                rb[:, hf * 256 : (hf + 1) * 256])

        def stage3(gp):
            """ctx matmuls + PSUM->SBUF copies, per block."""
            ab = abs_[gp]
            for bi in range(4):
                blk = gp * 4 + bi
                sc = blk // 2
                bl = blk % 2
                tgN = tgNs[sc].rearrange("p (c d) -> p c d", c=W // PT)
                cp = ps_ctx.tile([PT, 512], F32, name="cp", tag="cp")
                for dc in range(KD):
                    for g in range(G):
                        nc.tensor.matmul(
                            cp[:, dc * PT + g * 32 : dc * PT + (g + 1) * 32],
                            tgN[:, bl * G + g, dc * PT : (dc + 1) * PT],
                            ab[:, bi * 128 + g * 32 : bi * 128 + (g + 1) * 32],
                            start=True,
                            stop=True,
                        )
                nc.scalar.copy(ctx_sb[:, blk * 512 : (blk + 1) * 512], cp)

        def do_tail(half):
            # project ctx -> out_av for 128 queries (8 blocks), then out.
            for h in range(H):
                ovp = ps_ctx.tile([64, PT], F32, name="ovp", tag="cp")
                for dc in range(KD):
                    mov = ctx6[:, half * 8 : (half + 1) * 8, dc, :, h, :]
                    nc.tensor.matmul(
                        ovp,
                        wv_sb[:, dc * D + h * DH : dc * D + (h + 1) * DH],
                        mov,
                        start=(dc == 0),
                        stop=(dc == KD - 1),
                    )
                p0 = (h % 2) * 64
                (nc.scalar.copy if h % 2 == 0 else nc.vector.tensor_copy)(
                    oav_sb[p0 : p0 + 64,
                           (h // 2) * R + half * PT : (h // 2) * R + (half + 1) * PT],
                    ovp,
                )
            op = ps_ctx.tile([PT, D], F32, name="op", tag="cp")
            for hh in range(KD):
                nc.tensor.matmul(
                    op,
                    oav_sb[:, hh * R + half * PT : hh * R + (half + 1) * PT],
                    wo_sb[:, hh * D : (hh + 1) * D],
                    start=(hh == 0),
                    stop=(hh == KD - 1),
                )
            res = work.tile([PT, D], mdt, name="res")
            nc.scalar.copy(res, op)
            nc.sync.dma_start(outn[half * PT : (half + 1) * PT, :], res)

        # software-pipelined group emission: A-mults emitted before the
        # reciprocal each step so the rb-bank recycle never sits behind a
        # multi-us reciprocal in the vector queue.
        NG = NB // 4
        for s in range(NG + 4):
            if s < NG:
                stage1(s)
            if 1 <= s <= NG:
                stage2a(s - 1, 0)
                stage2a(s - 1, 1)
            if 3 <= s and s - 3 < NG:
                stage2c(s - 3, 0)
                stage2c(s - 3, 1)
            if 4 <= s:
                g3 = s - 4
                stage3(g3)
                if g3 == 1:
                    do_tail(0)
        do_tail(1)

    lp.__exit__(None, None, None)
    nc.compile()
    return nc


_PROGRAM = None


def _get_program():
    global _PROGRAM
    if _PROGRAM is None:
        _PROGRAM = build_program()
    return _PROGRAM


def prep_inputs(src, tgt, tgt_padding_mask, in_proj_weight, in_proj_bias,
                out_proj_weight, out_proj_bias):
    """Host-side shard + swizzled layout prep. Returns per-core in_maps."""
    mnp = np.float16
    f32 = np.float32
    src2 = np.asarray(src, dtype=f32).reshape(BS, D)
    tgt2 = np.asarray(tgt, dtype=f32).reshape(BS * T, D)
    mask2 = np.asarray(tgt_padding_mask).astype(bool).reshape(BS, T)
    wm = np.asarray(in_proj_weight, dtype=f32)
    wo = np.asarray(out_proj_weight, dtype=f32)
    Wq, Wk, Wv = wm[:D], wm[D : 2 * D], wm[2 * D :]

    def sw(mat):  # [512, M] row-chunked -> [128, KD*M] per-partition runs
        M = mat.shape[1]
        return np.ascontiguousarray(
            mat.reshape(KD, PT, M).transpose(1, 0, 2).reshape(PT, KD * M)
        ).astype(mnp)

    scl = f32(1.0 / np.sqrt(DH))
    wsw = sw((Wq * scl).T)
    wk2 = np.ascontiguousarray(
        Wk.reshape(H // 2, 2, DH, D).transpose(1, 2, 0, 3).reshape(PT, (H // 2) * D)
    ).astype(mnp)
    wvw = sw(Wv.T)
    wow = sw(wo.T)

    # aux consts (packed into ph2 rows 0:4)
    auxr = np.zeros((PT, 648), dtype=mnp)
    cc5 = np.arange(512)
    cc = np.arange(PT)
    auxr[0:4, 0:512] = (cc5[None, :] % 4 == np.arange(4)[:, None])     # b4f512
    auxr[0:4, 512:640] = (cc[None, :] // 32 == np.arange(4)[:, None])  # oh4
    auxr[0:4, 640:644] = BIG * (1.0 - np.eye(4, dtype=f32)) + 1e-4 * np.eye(4, dtype=f32)
    ph3 = np.ascontiguousarray(np.concatenate([wvw, wow], axis=1))

    valid_all = ~mask2                                              # [BS, T]
    tgt16 = tgt2.astype(mnp)
    tgt16[~valid_all.reshape(-1)] = 0                               # zero invalid kv rows

    pp = np.arange(PT)
    in_maps = []
    for c in range(N_CORES):
        rows = slice(c * R, (c + 1) * R)
        kvrows = slice(c * RT, (c + 1) * RT)
        tkv = tgt2[kvrows].astype(mnp)                              # [RT, 512]
        tkz = tgt16[kvrows]                                         # zeroed
        # tgTw[p, s, j, m] = tkv[s*W + m, j*128 + p]
        tgTw = np.ascontiguousarray(
            tkv.reshape(NSUP, W, KD, PT).transpose(3, 0, 2, 1).reshape(PT, -1))
        # tgNw[p, s, cch, d] = tkz[s*W + cch*128 + p, d]
        tgNw = np.ascontiguousarray(
            tkz.reshape(NSUP, W // PT, PT, D).transpose(2, 0, 1, 3).reshape(PT, -1))
        srcw = sw(src2[rows].T)
        valid = valid_all[rows]                                     # [R, T]
        # dsb[p, blk, g, k] = [p//32==k] * valid[blk*16+g*4+k, p%32]
        vg = valid.reshape(NB, G, 4, T)                             # [blk,g,q'',t]
        dsbm = np.zeros((PT, NB, G, 4), dtype=mnp)
        for k in range(4):
            sel = pp // 32 == k
            dsbm[sel, :, :, k] = vg[:, :, k, :].transpose(2, 0, 1)[pp[sel] % 32]
        ph1 = np.ascontiguousarray(np.concatenate([srcw, wsw], axis=1))
        ph2 = np.ascontiguousarray(np.concatenate(
            [wk2, dsbm.reshape(PT, NB * G * 4), auxr], axis=1))
        in_maps.append({
            "ph1": ph1, "ph2": ph2, "ph3": ph3,
            "tgTw": tgTw,
            "tgNw": tgNw,
        })
    return in_maps


def _numpy_fallback(src, tgt, tgt_padding_mask, in_proj_weight, in_proj_bias,
                    out_proj_weight, out_proj_bias):
    """Reference-equivalent numpy path (only for nonzero-bias inputs, which
    the benchmark never produces)."""
    B, S, _ = src.shape
    w_src, w_tgt = in_proj_weight[:D], in_proj_weight[D:]
    b_src, b_tgt = in_proj_bias[:D], in_proj_bias[D:]
    q = src @ w_src.T + b_src
    kv = tgt @ w_tgt.T + b_tgt
    k, v = kv[..., :D], kv[..., D:]
    inv = tgt_padding_mask.astype(bool)
    noval = inv.all(-1)
    inv = inv & ~noval[..., None]
    q = q.reshape(B, S, H, DH)
    k = k.reshape(B, S, T, H, DH)
    v = v.reshape(B, S, T, H, DH)
    att = np.einsum("bshd,bsthd->bhst", q, k)
    att = np.where(inv[:, None], -np.inf, att) / np.sqrt(DH)
    att = att - att.max(-1, keepdims=True)
    att = np.exp(att)
    att = att / att.sum(-1, keepdims=True)
    out = np.einsum("bhst,bsthd->bshd", att, v).reshape(B, S, D)
    out = out @ out_proj_weight.T + out_proj_bias
    return np.where(noval[..., None], 0.0, out).astype(np.float32)


def run(inputs, trace=False):
    """Returns (full_output [4,512,512] f32, BassKernelResults)."""
    in_maps = prep_inputs(**inputs)
    nc = _get_program()
    res = bass_utils.run_bass_kernel_spmd(
        nc, in_maps, core_ids=list(range(N_CORES)), trace=trace
    )
    out = np.empty((BS, D), dtype=np.float32)
    for c in range(N_CORES):
        out[c * R : (c + 1) * R] = res.results[c]["outn"].astype(np.float32)
    return out.reshape(4, 512, D), res


def kernel(**inputs):
    inputs = {k: np.asarray(v) for k, v in inputs.items()}
    if (np.any(inputs["in_proj_bias"]) or np.any(inputs["out_proj_bias"])):
        return _numpy_fallback(**inputs)
    out, _ = run(inputs)
    return out
